# revision 3
# baseline (speedup 1.0000x reference)
"""Multi-head attention + output projection (nn_AttentionBase) on 8 Trainium2
NeuronCores.

Reference computation (B=2, S=2048, E=2048, H=16, c=128, fp32):
    scores  = einsum('bqhc,bkhc->bhqk', q/sqrt(c), k)
    weights = softmax(scores + mask_bias_on_keys)
    out     = einsum('bhqk,bkhc->bqhc', weights, v) @ w_out.T

Sharding: 8 cores = (batch b: 2) x (query block of 512: 4). Each core computes
all 16 heads for its 512 queries against the valid keys of its batch, then
applies the full output projection for its rows. No inter-core reduction is
needed; the host concatenates the 8 [512, 2048] results.

Mask sparsity: the attention mask is a padding mask on keys (~50% masked).
Softmax+attention are permutation-invariant over keys, so the host sorts each
batch's keys (and values) valid-first. The kernel then only processes the
first NCHE = ceil(max_valid/128) key chunks; fully-masked tail chunks
contribute exactly zero (exp(-30000) == 0 in fp32) and are skipped. Chunks
below NFULL = min_b(valid_b)//128 are valid for BOTH batches, so their exp
runs with a constant zero bias and can be batched across PSUM banks; only the
few boundary chunks need the per-partition mask bias.

Per-core dataflow (matmuls bf16 with fp32 PSUM accumulation):
  scoresT[sk,sq] = kT.T @ qT            (per 128-key chunk, PE)
  pT = exp(scoresT * c^-0.5 [+ maskb])  (ScalarE; zero-bias chunks batched two
                                         per ACT over a 2-bank PSUM group to
                                         amortize the ~352-cycle ACT overhead)
  attnT[c,sq]   += v_chunk.T @ pT       (PE, accumulated over key chunks)
  s_pt[sk%128,sq] = sum_j pT_j          (GpSimd tensor adds - off the PE)
  l[1,sq]        = ones.T @ s_pt        (PE, ONE M=1 matmul per head)
  attn_sb[c,sq]  = attnT * (1/l)        (VectorE; approx reciprocal + 1/l
                                         partition-broadcast via a DRAM bounce)
  y[sq,e_out]    = sum_h attn_sb_h.T @ w_outT  (PE, contraction over e_in)
"""
import sys

sys.path.insert(0, "/opt/trn_rl_repo")

import math

import ml_dtypes
import numpy as np

import concourse.bass as bass
import concourse.mybir as mybir
import concourse.tile as tile

B, S, E = 2, 2048, 2048
H, C = 16, 128
SQ = 512          # queries per core
NSQT = SQ // 128   # 4 query subtiles
NNT = E // 512     # 4 output column tiles
MASK_NEG = -30000.0
BF16 = mybir.dt.bfloat16
F32 = mybir.dt.float32


_WAIT_LIMIT = 1


def _split_excess_waits(nc, limit=_WAIT_LIMIT):
    """The walrus build in this container rejects instructions carrying more
    than one semaphore wait ("Too many sync wait commands"). Move excess waits
    onto NoOps inserted just before the instruction on the same engine (engine
    streams execute in block order, so the waits still gate the instruction)."""
    for f in nc.m.functions:
        for bb in f.blocks:
            new = []
            changed = False
            for inst in bb.instructions:
                si = inst.sync_info
                if si is not None and len(si.on_wait) > limit:
                    waits = list(si.on_wait)
                    excess, keep = waits[:-limit], waits[-limit:]
                    for k in range(0, len(excess), limit):
                        nop = mybir.InstNoOp(
                            name=f"{inst.name}-wsplit{k}",
                            sync_info=mybir.SyncInfo(
                                on_wait=excess[k:k + limit], on_update=[]
                            ),
                            bass_nofuse=True,
                            engine=inst.engine,
                        )
                        new.append(nop)
                    inst.sync_info = mybir.SyncInfo(
                        on_wait=keep, on_update=list(si.on_update)
                    )
                    changed = True
                new.append(inst)
            if changed:
                bb.instructions = new


def _build_program(nfull, nche):
    """Build the per-core program for nche key chunks, of which the first
    nfull are fully valid for both batches (constant zero mask bias)."""
    nc = bass.Bass()
    nk = nche * 128
    qT = nc.declare_dram_parameter("qT", [H, C, SQ], BF16, isOutput=False)
    kT = nc.declare_dram_parameter("kT", [H, C, nk], BF16, isOutput=False)
    v = nc.declare_dram_parameter("v", [H, 128, nche, C], BF16, isOutput=False)
    wT = nc.declare_dram_parameter("wT", [E, E], BF16, isOutput=False)
    maskb = nc.declare_dram_parameter("maskb", [128, nche], F32, isOutput=False)
    y = nc.declare_dram_parameter("y", [SQ, E], F32, isOutput=True)

    scale = 1.0 / math.sqrt(C)

    # ACT groups: (chunk0, nchunks, zero_bias)
    groups = []
    for g in range(nfull // 2):
        groups.append((2 * g, 2, True))
    if nfull % 2:
        groups.append((nfull - 1, 1, True))
    for j in range(nfull, nche):
        groups.append((j, 1, False))

    with tile.TileContext(nc) as tc:
        with (
            tc.tile_pool(name="consts", bufs=1) as consts,
            tc.tile_pool(name="wpool", bufs=1) as wpool,
            tc.tile_pool(name="attn_all", bufs=1) as attn_all,
            tc.tile_pool(name="kv", bufs=2) as kv,
            tc.tile_pool(name="pt", bufs=6) as ptpool,
            tc.tile_pool(name="spt", bufs=2) as sptpool,
            tc.tile_pool(name="small", bufs=2) as small,
            tc.tile_pool(name="lbc", bufs=4) as lbc,
            tc.tile_pool(name="raw", bufs=3) as rawpool,
            tc.tile_pool(name="ldram", bufs=2, space="DRAM") as ldram,
            tc.tile_pool(name="yout", bufs=3) as yout,
            tc.tile_pool(name="psS", bufs=2, space="PSUM") as psS,
            tc.tile_pool(name="psA", bufs=2, space="PSUM") as psA,
            tc.tile_pool(name="psL", bufs=2, space="PSUM") as psL,
        ):
            ones = consts.tile([128, 1], F32)
            nc.vector.memset(ones, 1.0)
            maskb_sb = consts.tile([128, nche], F32)
            nc.sync.dma_start(maskb_sb, maskb[:, :])

            w_sb = wpool.tile([128, E // 128, E], BF16)
            attn_tiles = [attn_all.tile([128, SQ], BF16, tag=f"a{h}",
                                        name=f"attn{h}") for h in range(H)]

            for h in range(H):
                # K^T loaded per ACT group: the first scores matmul only
                # waits on its own group's columns.
                kts = [kv.tile([128, n * 128], BF16, tag=f"kt{gi}",
                               name=f"kt{gi}")
                       for gi, (_, n, _) in enumerate(groups)]
                qt = kv.tile([128, SQ], BF16, tag="qt")
                nc.sync.dma_start(qt, qT[h])
                for gi, (c0, n, _) in enumerate(groups):
                    nc.sync.dma_start(
                        kts[gi], kT[h][:, c0 * 128:(c0 + n) * 128]
                    )
                vt = kv.tile([128, nche, C], BF16, tag="vt")
                nc.sync.dma_start(vt, v[h])
                # head h's slice of the projection weights, used in phase B
                nc.sync.dma_start(w_sb[:, h, :], wT[h * 128:(h + 1) * 128, :])

                ps_at = psA.tile([128, SQ], F32)
                ps_l = psL.tile([1, SQ], F32, tag="ly")

                # scores + exp per group; zero-bias groups batch 2 chunks
                # into one ACT over a 2-bank PSUM tile.
                pt_slices = []  # chunk j -> AP of its [128, SQ] exp tile
                for gi, (c0, n, zb) in enumerate(groups):
                    ps_g = psS.tile([128, n * SQ], F32)
                    for jj in range(n):
                        nc.tensor.matmul(
                            ps_g[:, jj * SQ:(jj + 1) * SQ],
                            lhsT=kts[gi][:, jj * 128:(jj + 1) * 128],
                            rhs=qt,
                            start=True, stop=True,
                        )
                    pt_g = ptpool.tile([128, n * SQ], BF16)
                    bias = 0.0 if zb else maskb_sb[:, c0:c0 + 1]
                    nc.scalar.activation(
                        pt_g, ps_g, mybir.ActivationFunctionType.Exp,
                        bias=bias, scale=scale,
                    )
                    for jj in range(n):
                        pt_slices.append(pt_g[:, jj * SQ:(jj + 1) * SQ])

                for j in range(nche):
                    nc.tensor.matmul(
                        ps_at, lhsT=vt[:, j, :], rhs=pt_slices[j],
                        start=(j == 0), stop=(j == nche - 1),
                    )

                # softmax denominator: accumulate the exp tiles on GpSimd
                # (otherwise idle), then a single M=1 matmul with ones.
                s_pt = sptpool.tile([128, SQ], F32)
                if nche == 1:
                    nc.gpsimd.tensor_copy(s_pt, pt_slices[0])
                else:
                    nc.gpsimd.tensor_add(s_pt, pt_slices[0], pt_slices[1])
                    for j in range(2, nche):
                        nc.gpsimd.tensor_add(s_pt, s_pt, pt_slices[j])
                nc.tensor.matmul(ps_l, lhsT=ones, rhs=s_pt,
                                 start=True, stop=True)

                # Free both PSUM slots with fast DVE ops; the 1/l
                # normalization (DRAM-bounce partition broadcast) runs off
                # the critical path, before phase B reads attn_tiles[h].
                araw = rawpool.tile([128, SQ], F32)
                nc.vector.tensor_copy(araw, ps_at)
                lr = small.tile([1, SQ], F32)
                nc.vector.reciprocal(lr, ps_l)
                ld = ldram.tile([1, SQ], F32)
                nc.sync.dma_start(ld, lr)
                lb = lbc.tile([128, SQ], F32)
                nc.sync.dma_start(
                    lb,
                    bass.AP(tensor=ld.tensor, offset=ld.offset,
                            ap=[[0, 128]] + list(ld.ap[1:])),
                )
                nc.gpsimd.tensor_mul(attn_tiles[h], araw, lb)

            for i in range(NSQT):
                for n in range(NNT):
                    ps_y = psL.tile([128, 512], F32, tag="ly")
                    for ec in range(H):
                        nc.tensor.matmul(
                            ps_y,
                            lhsT=attn_tiles[ec][:, i * 128:(i + 1) * 128],
                            rhs=w_sb[:, ec, n * 512:(n + 1) * 512],
                            start=(ec == 0), stop=(ec == H - 1),
                        )
                    yt = yout.tile([128, 512], F32)
                    nc.scalar.copy(yt, ps_y)
                    nc.sync.dma_start(
                        y[i * 128:(i + 1) * 128, n * 512:(n + 1) * 512], yt
                    )

    _split_excess_waits(nc)
    return nc


_PROGRAMS = {}


def _get_program(nfull, nche):
    key = (nfull, nche)
    if key not in _PROGRAMS:
        _PROGRAMS[key] = _build_program(nfull, nche)
    return _PROGRAMS[key]


def _make_in_maps(keys, values, queries, attention_mask, w_out):
    bf = ml_dtypes.bfloat16
    wT_host = np.ascontiguousarray(w_out.astype(bf).T)

    nv = attention_mask.sum(axis=1).astype(np.int64)  # valid keys per batch
    nfull = int(nv.min()) // 128
    nche = max(1, int(-(-int(nv.max()) // 128)))
    nk = nche * 128

    per_batch = []
    for b in range(B):
        order = np.argsort(~attention_mask[b], kind="stable")[:nk]
        kb = keys[b][order].astype(bf).reshape(nk, H, C)
        kT_host = np.ascontiguousarray(kb.transpose(1, 2, 0))
        vb = values[b][order].astype(bf).reshape(nche, 128, H, C)
        v_host = np.ascontiguousarray(vb.transpose(2, 1, 0, 3))
        mb = np.where(attention_mask[b][order], 0.0, MASK_NEG).astype(np.float32)
        maskb_host = np.ascontiguousarray(mb.reshape(nche, 128).T)
        per_batch.append((kT_host, v_host, maskb_host))

    in_maps = []
    for core in range(8):
        b = core // 4
        q0 = (core % 4) * SQ
        qb = queries[b, q0:q0 + SQ].astype(bf).reshape(SQ, H, C)
        qT_host = np.ascontiguousarray(qb.transpose(1, 2, 0))
        kT_host, v_host, maskb_host = per_batch[b]
        in_maps.append({
            "qT": qT_host,
            "kT": kT_host,
            "v": v_host,
            "wT": wT_host,
            "maskb": maskb_host,
        })
    return in_maps, nfull, nche


def _run(inputs, trace=False, trace_cores=None):
    from concourse.bass_utils import run_bass_kernel_spmd

    in_maps, nfull, nche = _make_in_maps(**inputs)
    nc = _get_program(nfull, nche)
    res = run_bass_kernel_spmd(
        nc, in_maps, core_ids=list(range(8)),
        trace=trace, trace_cores=trace_cores,
    )
    out = np.empty((B, S, E), dtype=np.float32)
    for core in range(8):
        b = core // 4
        q0 = (core % 4) * SQ
        out[b, q0:q0 + SQ, :] = res.results[core]["y"]
    return out, res


def kernel(keys, values, queries, attention_mask, w_out):
    out, _ = _run(dict(
        keys=np.asarray(keys), values=np.asarray(values),
        queries=np.asarray(queries),
        attention_mask=np.asarray(attention_mask),
        w_out=np.asarray(w_out),
    ))
    return out
